# revision 3
# baseline (speedup 1.0000x reference)
"""Trainium2 Bass kernel for nn_BartCrossAttention (B=4, L=1024, D=1024, H=16, HD=64).

Sharding: 8 cores; core c handles query tokens [512c, 512c+512) (batch b = c//2).
Each core recomputes K/V projections for its *whole* batch (1024 kv tokens) so no
collective is needed; the host slices inputs per core and concatenates outputs.

v2: all weights/activations in bf16 (same 1 cycle/row PE rate as fp32r, half the
DMA bytes and SBUF footprint; fp32 PSUM accumulation keeps the error ~1e-3).
All four weight matrices are preloaded into SBUF as whole tensors (big efficient
DMAs, no in-loop weight fetch). Softmax normalization is done per head-pair
entirely in SBUF (reciprocal + gpsimd partition_broadcast + multiply fused into
the PSUM evict) instead of the old DRAM round-trip chain, so the out-projection
starts immediately after the last pair's attention.

Per-core dataflow (activations kept in [feature, token] i.e. transposed layout so
every matmul contracts over the partition dim):
  prologue: PE-transpose kv slice -> kvT; V = kvT_tile.T @ Wv (+ones column per
            head block for fused softmax denominators); PE-transpose hidden
  per head-pair hp (interleaved so PE never starves while ACT runs exp):
    K^T(hp) = Wk_tile.T @ kvT;  Q^T(hp) = Wq_tile.T @ hidT (Wq pre-scaled 1/8)
    per kpos tile: S^T = K^T_h.T @ Q^T_h; attn = exp(S^T) (no max-subtraction:
    scores are O(9) for this data, exp safe in fp32);
    matmul(lhsT=[V_h|1], rhs=attn) accumulated -> rows 0..63 ctx^T, row 64 sums
    then: recip(sums) in SBUF, broadcast, ctx^T *= recip fused with PSUM evict
  epilogue: out = ctxT_tile.T @ Wo + out_bias
"""
import sys

for _p in ("/opt/trn_rl_repo",):
    if _p not in sys.path:
        sys.path.insert(0, _p)

import numpy as np
import ml_dtypes

import concourse.bass as bass
import concourse.mybir as mybir
import concourse.tile as tile
from concourse import bacc
import concourse.bass_utils as bass_utils
from concourse.masks import make_identity

F32 = mybir.dt.float32
BF16 = mybir.dt.bfloat16

P = 128
D = 1024        # model dim
H = 16          # heads
NCORES = 8
TQ = 512        # query tokens per core
LK = 1024       # kv tokens per batch
B, LQ = 4, 1024

_CACHE = {}


def _build_core_program():
    nc = bacc.Bacc("TRN2", target_bir_lowering=False, debug=False,
                   num_devices=NCORES)

    hid_s = nc.dram_tensor("hid_s", [TQ, D], BF16, kind="ExternalInput")
    kv_s = nc.dram_tensor("kv_s", [LK, D], BF16, kind="ExternalInput")
    wq_t = nc.dram_tensor("wq_t", [D, D], BF16, kind="ExternalInput")
    wk_t = nc.dram_tensor("wk_t", [D, D], BF16, kind="ExternalInput")
    wv_t = nc.dram_tensor("wv_t", [D, D], BF16, kind="ExternalInput")
    wo_t = nc.dram_tensor("wo_t", [D, D], BF16, kind="ExternalInput")
    qb_d = nc.dram_tensor("qb", [D], F32, kind="ExternalInput")
    kb_d = nc.dram_tensor("kb", [D], F32, kind="ExternalInput")
    vb_d = nc.dram_tensor("vb", [D], F32, kind="ExternalInput")
    ob_d = nc.dram_tensor("ob", [D], F32, kind="ExternalInput")
    out_s = nc.dram_tensor("out_s", [TQ, D], F32, kind="ExternalOutput")

    Exp = mybir.ActivationFunctionType.Exp
    Ident = mybir.ActivationFunctionType.Identity
    add = mybir.AluOpType.add
    mult = mybir.AluOpType.mult

    with tile.TileContext(nc) as tc:
        with (
            tc.tile_pool(name="setup", bufs=1) as setup,
            tc.tile_pool(name="big", bufs=1) as big,
            tc.tile_pool(name="attn", bufs=2) as attnp,
            tc.tile_pool(name="wfull", bufs=1) as wfull,
            tc.tile_pool(name="psmm", bufs=2, space="PSUM") as psmm,
            tc.tile_pool(name="rb", bufs=2) as rbp,
        ):
            # ---- setup: identity, biases ----
            identF = setup.tile([P, P], F32, tag="identF")
            make_identity(nc, identF[:])
            ident = setup.tile([P, P], BF16, tag="ident")
            nc.vector.tensor_copy(ident[:], identF[:])

            qb_sb = setup.tile([P, 8], F32, tag="qb")
            nc.sync.dma_start(qb_sb[:], qb_d.ap().rearrange("(o p) -> p o", p=P))
            kb_sb = setup.tile([P, 8], F32, tag="kb")
            nc.sync.dma_start(kb_sb[:], kb_d.ap().rearrange("(o p) -> p o", p=P))
            vbB = setup.tile([P, D], F32, tag="vbB")
            obB = setup.tile([P, D], F32, tag="obB")

            def load_w_full(dram, tag):
                # [D, D] -> tile [128, 8, 1024]: w[di*128+p, o] at [p, di, o]
                t = wfull.tile([P, 8, D], BF16, tag=tag)
                nc.sync.dma_start(
                    t[:], dram.ap().rearrange("(dd p) o -> p dd o", p=P))
                return t

            # ---- persistent big tiles ----
            KT = big.tile([P, 8, LK], BF16, tag="KT")        # K^T [1024, 1024]
            v65 = big.tile([P, 8, H * 65], BF16, tag="v65")  # V+ones [1024,1040]
            qT = big.tile([P, 8, TQ], BF16, tag="qT")        # Q^T [1024, 512]
            ctxT = big.tile([P, 8, TQ], BF16, tag="ctxT")    # ctx^T [1024, 512]

            # ones columns of v65 (col 64 of each head block)
            onesF = setup.tile([P, P], F32, tag="identF")
            nc.gpsimd.memset(onesF[:], 1.0)
            nc.vector.tensor_copy(
                v65[:].rearrange("p t (h x) -> p t h x", x=65)[:, :, :, 64:65],
                onesF[:].rearrange("p (t h x) -> p t h x", t=8, h=16))

            with tc.tile_pool(name="xTp", bufs=1) as xTp:
                kvT = xTp.tile([P, 8, LK], BF16, tag="kvT")   # kv^T [D, 1024]
                hidT = xTp.tile([P, 8, TQ], BF16, tag="hidT")  # hid^T [1024,512]

                with (
                    tc.tile_pool(name="xn", bufs=2) as xn,
                    tc.tile_pool(name="wvpool", bufs=1) as wvpool,
                    tc.tile_pool(name="pst", bufs=2, space="PSUM") as pst,
                ):
                    # bias rows -> broadcast
                    vb_row = xn.tile([1, D], F32, tag="xn")
                    nc.sync.dma_start(vb_row[:], vb_d.ap()[None, :])
                    nc.gpsimd.partition_broadcast(vbB[:], vb_row[:])
                    ob_row = xn.tile([1, D], F32, tag="xn")
                    nc.sync.dma_start(ob_row[:], ob_d.ap()[None, :])
                    nc.gpsimd.partition_broadcast(obB[:], ob_row[:])

                    # transposes: src [ntt*128, D] natural -> dst [128,8,ntt*128]
                    def transpose_in(dst, src_dram, ntt):
                        for tt in range(ntt):
                            for dhalf in range(2):
                                nsrc = xn.tile([P, 512], BF16, tag="xn")
                                nc.sync.dma_start(
                                    nsrc[:],
                                    src_dram.ap().rearrange(
                                        "(tt p) d -> p tt d", p=P)[
                                        :, tt, dhalf * 512:(dhalf + 1) * 512],
                                )
                                for dq in range(2):
                                    dh = dhalf * 2 + dq
                                    tp = pst.tile([P, 256], BF16, tag="tp")
                                    for dl in range(2):
                                        di = dq * 2 + dl
                                        nc.tensor.transpose(
                                            tp[:, dl * P:(dl + 1) * P],
                                            nsrc[:, di * P:(di + 1) * P],
                                            ident[:],
                                        )
                                    if dh % 2 == 0:
                                        nc.scalar.activation(
                                            dst[:, 2 * dh, tt * P:(tt + 1) * P],
                                            tp[:, 0:P], Ident)
                                        nc.scalar.activation(
                                            dst[:, 2 * dh + 1,
                                                tt * P:(tt + 1) * P],
                                            tp[:, P:2 * P], Ident)
                                    else:
                                        nc.vector.tensor_copy(
                                            dst[:, 2 * dh, tt * P:(tt + 1) * P],
                                            tp[:, 0:P])
                                        nc.vector.tensor_copy(
                                            dst[:, 2 * dh + 1,
                                                tt * P:(tt + 1) * P],
                                            tp[:, P:2 * P])

                    # ---- prologue: kv transposes first (kv chunks get the
                    # DMA queue ahead of the wv bytes), then V projection ----
                    transpose_in(kvT, kv_s, 8)
                    wv = wvpool.tile([P, 8, D], BF16, tag="wv")
                    nc.sync.dma_start(
                        wv[:], wv_t.ap().rearrange("(dd p) o -> p dd o", p=P))

                    for half in range(2):             # v-col half
                        if half == 1:
                            # hid transposes fill the PE while wv half 1 loads
                            transpose_in(hidT, hid_s, 4)
                        for ti in range(8):           # kv token tile
                            pp = psmm.tile([P, 512], F32, tag="pp")
                            for di in range(8):
                                nc.tensor.matmul(
                                    pp[:],
                                    kvT[:, di, ti * P:(ti + 1) * P],
                                    wv[:, di, half * 512:(half + 1) * 512],
                                    start=(di == 0), stop=(di == 7),
                                )
                            dst = v65[:].rearrange(
                                "p t (h x) -> p t h x", x=65)[
                                :, ti, half * 8:(half + 1) * 8, 0:64]
                            nc.vector.tensor_tensor(
                                dst, pp[:],
                                vbB[:, half * 512:(half + 1) * 512], add)

                # ---- main loop: per head-pair K/Q projection + attention ----
                with (
                    tc.tile_pool(name="psctx", bufs=2, space="PSUM") as psctx,
                    tc.tile_pool(name="pssc2", bufs=2, space="PSUM") as pssc2,
                ):
                    wk = load_w_full(wk_t, "wk")
                    wq = load_w_full(wq_t, "wq")
                    wo = load_w_full(wo_t, "wo")

                    def emit_kproj(hp, nk):
                        pp = psmm.tile([P, 512], F32, tag="pp",
                                       name=f"ppk{hp}_{nk}")
                        for di in range(8):
                            nc.tensor.matmul(
                                pp[:],
                                wk[:, di, hp * P:(hp + 1) * P],
                                kvT[:, di, nk * 512:(nk + 1) * 512],
                                start=(di == 0), stop=(di == 7),
                            )
                        nc.vector.tensor_scalar(
                            KT[:, hp, nk * 512:(nk + 1) * 512], pp[:],
                            kb_sb[:, hp:hp + 1], None, add)

                    def emit_qproj(hp):
                        pq = psmm.tile([P, 512], F32, tag="pp",
                                       name=f"ppq{hp}")
                        for di in range(8):
                            nc.tensor.matmul(
                                pq[:],
                                wq[:, di, hp * P:(hp + 1) * P],
                                hidT[:, di, :],
                                start=(di == 0), stop=(di == 7),
                            )
                        nc.vector.tensor_scalar(qT[:, hp, :], pq[:],
                                                qb_sb[:, hp:hp + 1], None, add)

                    # pair 0 projections up front
                    emit_kproj(0, 0)
                    emit_kproj(0, 1)
                    emit_qproj(0)

                    for hp in range(8):
                        nxt = hp + 1
                        ctx_ps = [psctx.tile([65, 512], F32, tag="ctx",
                                             name=f"ctx{hp}_{i}")
                                  for i in range(2)]
                        for t in range(8):
                            sc2 = pssc2.tile([P, 1024], F32, tag="sc2",
                                            name=f"sc2_{hp}_{t}")
                            for hh in range(2):
                                lo = 64 * hh
                                nc.tensor.matmul(
                                    sc2[:, hh * 512:(hh + 1) * 512],
                                    KT[lo:lo + 64, hp, t * P:(t + 1) * P],
                                    qT[lo:lo + 64, hp, :],
                                    start=True, stop=True,
                                )
                            at2 = attnp.tile([P, 1024], BF16, tag="at")
                            nc.scalar.activation(at2[:], sc2[:], Exp)
                            for hh in range(2):
                                h = 2 * hp + hh
                                nc.tensor.matmul(
                                    ctx_ps[hh][:],
                                    v65[:, t, h * 65:(h + 1) * 65],
                                    at2[:, hh * 512:(hh + 1) * 512],
                                    start=(t == 0), stop=(t == 7),
                                )
                            if nxt < 8:
                                if t == 1:
                                    emit_kproj(nxt, 0)
                                elif t == 3:
                                    emit_kproj(nxt, 1)
                                elif t == 5:
                                    emit_qproj(nxt)
                        # per-pair softmax normalization, all in SBUF:
                        # recip of the fused denominator row, broadcast down
                        # 64 partitions, multiply fused with the PSUM evict.
                        srow = rbp.tile([1, 1024], F32, tag="srow",
                                        name=f"srow{hp}")
                        nc.vector.tensor_copy(srow[:, 0:512],
                                              ctx_ps[0][64:65, :])
                        nc.vector.tensor_copy(srow[:, 512:1024],
                                              ctx_ps[1][64:65, :])
                        nc.vector.reciprocal(srow[:], srow[:])
                        rcp = rbp.tile([64, 1024], F32, tag="rcp",
                                       name=f"rcp{hp}")
                        nc.gpsimd.partition_broadcast(rcp[:], srow[:])
                        nc.vector.tensor_tensor(
                            ctxT[0:64, hp, :], ctx_ps[0][0:64, :],
                            rcp[:, 0:512], mult)
                        nc.vector.tensor_tensor(
                            ctxT[64:128, hp, :], ctx_ps[1][0:64, :],
                            rcp[:, 512:1024], mult)

            # ---- epilogue: out projection ----
            with tc.tile_pool(name="outp", bufs=2) as outp:
                for half in range(2):
                    for mi in range(4):
                        ot = outp.tile([P, 512], F32, tag="ot")
                        po = psmm.tile([P, 512], F32, tag="pp")
                        for fj in range(8):
                            nc.tensor.matmul(
                                po[:],
                                ctxT[:, fj, mi * P:(mi + 1) * P],
                                wo[:, fj, half * 512:(half + 1) * 512],
                                start=(fj == 0), stop=(fj == 7),
                            )
                        nc.vector.tensor_tensor(
                            ot[:], po[:],
                            obB[:, half * 512:(half + 1) * 512], add)
                        nc.sync.dma_start(
                            out_s.ap().rearrange("(mm p) d -> p mm d", p=P)[
                                :, mi, half * 512:(half + 1) * 512],
                            ot[:])

    nc.compile()
    return nc


def _prep_inputs(hidden_states, key_value_states, q_weight, q_bias,
                 kv_weight, kv_bias, out_weight, out_bias):
    f32 = np.float32
    bf16 = ml_dtypes.bfloat16
    hid = np.asarray(hidden_states, f32).reshape(B * LQ, D).astype(bf16)
    kv = np.asarray(key_value_states, f32).reshape(B * LK, D).astype(bf16)
    scale = f32(1.0 / 8.0)

    # de-interleave kv rows: row e <-> (h=e//128, j=(e%128)//64, d=e%64)
    e = np.arange(2 * D)
    kmask = (e % 128) < 64
    kidx, vidx = e[kmask], e[~kmask]
    kvw = np.asarray(kv_weight, f32)
    kvb = np.asarray(kv_bias, f32)

    shared = {
        "wq_t": np.ascontiguousarray(
            (np.asarray(q_weight, f32) * scale).T).astype(bf16),
        "wk_t": np.ascontiguousarray(kvw[kidx].T).astype(bf16),
        "wv_t": np.ascontiguousarray(kvw[vidx].T).astype(bf16),
        "wo_t": np.ascontiguousarray(np.asarray(out_weight, f32).T).astype(bf16),
        "qb": np.ascontiguousarray(np.asarray(q_bias, f32) * scale),
        "kb": np.ascontiguousarray(kvb[kidx]),
        "vb": np.ascontiguousarray(kvb[vidx]),
        "ob": np.ascontiguousarray(np.asarray(out_bias, f32)),
    }
    in_maps = []
    for c in range(NCORES):
        b = c // 2
        m = dict(shared)
        m["hid_s"] = np.ascontiguousarray(hid[c * TQ:(c + 1) * TQ])
        m["kv_s"] = np.ascontiguousarray(kv[b * LK:(b + 1) * LK])
        in_maps.append(m)
    return in_maps


def kernel(hidden_states, key_value_states, q_weight, q_bias,
           kv_weight, kv_bias, out_weight, out_bias, _trace=False):
    if "nc" not in _CACHE:
        _CACHE["nc"] = _build_core_program()
    nc = _CACHE["nc"]
    in_maps = _prep_inputs(hidden_states, key_value_states, q_weight, q_bias,
                           kv_weight, kv_bias, out_weight, out_bias)
    res = bass_utils.run_bass_kernel_spmd(
        nc, in_maps, core_ids=list(range(NCORES)), trace=_trace)
    _CACHE["last_result"] = res
    out = np.concatenate([r["out_s"] for r in res.results], axis=0)
    return out.reshape(B, LQ, D)
